# revision 18
# baseline (speedup 1.0000x reference)
"""Trainium2 Bass kernel for nn_BiInteraction.

Reference computation:
    x: [B=8192, N=34, D=16] f32, W: [D, D] f32
    proj = einsum('bnd,de->bne', x, W)
    pairs (i, j) for i in [0, N-2], j in [i, N-1]  -> P = 594 pairs
    out[:, p, :] = proj[:, i_p, :] * x[:, j_p, :]  -> reshape [B, P*D = 9504]

Sharding: data-parallel over batch, 1024 rows per core, 8 cores.

Per-core kernel (per 128-batch tile; all stages pipelined by Tile):
  1. DMA x tile [128, 544] (batch on partitions, (n,d) flattened free dim);
     all 8 x tiles are prefetched up front so input DMAs never queue
     behind output DMAs
  2. Per 128-col block c: TensorE transpose -> ScalarE copy to SBUF ->
     TensorE matmul(lhsT=xT_block, rhs=W_blockdiag) -> ScalarE copy, giving
     proj[b, (n e)] in batch-major layout.  W_blockdiag [128,128] has W on
     the 16x16 diagonal blocks, so the contraction over rows (n', d)
     reduces over d only, separately per field n.
  3. VectorE: pair-groups: group i covers output pairs (i, j) for
     j in [i, 33] — a contiguous x slice times a broadcast proj block.
     Two adjacent groups are fused into ONE tensor_mul via explicit
     [step, count] access patterns (overlapping x windows, group i+1
     padded to group i's width; the D-column garbage spill into group
     i+2's start is rewritten by the next pair before any DMA reads it).
  4. Output [128, 9504] is DMA'd in 5 column chunks as the pairs finish,
     so the store stream starts ~10us into the kernel and stays saturated.
"""

import numpy as np

import concourse.bacc as bacc
import concourse.tile as tile
import concourse.mybir as mybir
from concourse import masks
from concourse.bass_types import AP
from concourse.bass_utils import run_bass_kernel_spmd

B, N, D = 8192, 34, 16
NCORES = 8
BLOC = B // NCORES            # 1024 rows per core
PTILE = 128                   # batch rows per tile (SBUF partitions)
NTILES = BLOC // PTILE        # 8
F = N * D                     # 544
F_PAD = F + D                 # pair-TT overlap pad
NPAIR = N * (N + 1) // 2 - 1  # 594
FOUT = NPAIR * D              # 9504

# group i covers pairs (i, j) for j in [i, N-1]; GOFF[i] = first pair index
GOFF = [0] * (N - 1)
for _i in range(1, N - 1):
    GOFF[_i] = GOFF[_i - 1] + (N - _i + 1)

_CACHE = {}


def _build_nc(repeat: int = 1):
    nc = bacc.Bacc("TRN2", target_bir_lowering=False, debug=False,
                   num_devices=NCORES)
    x_in = nc.dram_tensor("x", [BLOC, F], mybir.dt.float32,
                          kind="ExternalInput").ap()
    w_in = nc.dram_tensor("w", [D, D], mybir.dt.float32,
                          kind="ExternalInput").ap()
    y_out = nc.dram_tensor("out", [BLOC, FOUT], mybir.dt.float32,
                           kind="ExternalOutput").ap()

    f32 = mybir.dt.float32
    with tile.TileContext(nc) as tc:
        with (
            tc.tile_pool(name="const", bufs=1) as const_pool,
            tc.tile_pool(name="x", bufs=8) as x_pool,
            tc.tile_pool(name="xT_ps", bufs=2, space="PSUM") as xT_ps_pool,
            tc.tile_pool(name="xT_sb", bufs=2) as xT_sb_pool,
            tc.tile_pool(name="proj_ps", bufs=2, space="PSUM") as proj_ps_pool,
            tc.tile_pool(name="proj_sb", bufs=2) as proj_sb_pool,
            tc.tile_pool(name="out", bufs=4) as out_pool,
        ):
            # constants first: the 8 tiny block-diagonal-W DMAs issue while
            # the DMA engines would be idle anyway, so W_blockdiag is ready
            # the moment tile 0's x lands
            wbd = const_pool.tile([128, 128], f32)
            nc.gpsimd.memset(wbd[:], 0.0)
            for n in range(8):
                nc.sync.dma_start(wbd[16 * n:16 * n + 16, 16 * n:16 * n + 16],
                                  w_in[:, :])

            xts = []
            xt0 = x_pool.tile([PTILE, F_PAD], f32, tag="xt")
            nc.sync.dma_start(xt0[:, 0:F], x_in[0:PTILE, :])
            xts.append(xt0)
            ident = const_pool.tile([128, 128], f32)
            masks.make_identity(nc, ident[:])
            # dummy copy pulls the one-time ACT table load off the
            # critical path
            warm = const_pool.tile([1, 2], f32)
            nc.gpsimd.memset(warm[:], 0.0)
            nc.scalar.copy(warm[0:1, 1:2], warm[0:1, 0:1])

            # prefetch remaining x tiles (x is tiny: 17KB/partition total)
            for t in range(1, NTILES):
                xt = x_pool.tile([PTILE, F_PAD], f32, tag="xt")
                nc.gpsimd.dma_start(xt[:, 0:F], x_in[t * PTILE:(t + 1) * PTILE, :])
                xts.append(xt)

            # output DMA split points (group indices); last chunk small to
            # shrink the pipeline tail
            SPLITS = [2, 4, 8, 16, 24]

            for t in range(repeat * NTILES):
                xt = xts[t % NTILES]
                row0 = (t % NTILES) * PTILE

                # per 128-col block c: transpose -> copy -> proj matmul ->
                # copy, so group TTs for fields 8c..8c+7 start early
                xT_ps = xT_ps_pool.tile([128, 5 * 128], f32)
                xT = xT_sb_pool.tile([128, 5 * 128], f32)
                proj_ps = proj_ps_pool.tile([PTILE, F], f32)
                proj = proj_sb_pool.tile([PTILE, F], f32)
                for c in range(4):
                    nc.tensor.transpose(xT_ps[:, 128 * c:128 * (c + 1)],
                                        xt[:, 128 * c:128 * (c + 1)],
                                        ident[:])
                    nc.scalar.copy(xT[:, 128 * c:128 * (c + 1)],
                                   xT_ps[:, 128 * c:128 * (c + 1)])
                    nc.tensor.matmul(proj_ps[:, 128 * c:128 * (c + 1)],
                                     lhsT=xT[:, 128 * c:128 * (c + 1)],
                                     rhs=wbd[:], start=True, stop=True)
                    nc.scalar.copy(proj[:, 128 * c:128 * (c + 1)],
                                   proj_ps[:, 128 * c:128 * (c + 1)])
                nc.tensor.transpose(xT_ps[0:32, 512:640], xt[:, 512:544],
                                    ident[:])
                nc.scalar.copy(xT[0:32, 512:640], xT_ps[0:32, 512:640])
                nc.tensor.matmul(proj_ps[:, 512:544],
                                 lhsT=xT[0:32, 512:640],
                                 rhs=wbd[0:32, 0:32], start=True, stop=True)
                nc.scalar.copy(proj[:, 512:544], proj_ps[:, 512:544])

                # pairwise products: one broadcast tensor_mul per PAIR of
                # groups (i, i+1), group i+1 padded to group i's width. The
                # pad overwrites the first D cols of group i+2 with garbage,
                # which the next pair's TT rewrites before any DMA (all
                # SPLITS are even groups). DMA out finished chunks as we go.
                out_t = out_pool.tile([PTILE, FOUT], f32)
                chunk_lo = 0
                for i in range(0, N - 1, 2):
                    w_cols = (N - i) * D     # padded per-group width
                    ng = 2 if i + 1 < N - 1 else 1
                    off = GOFF[i] * D
                    dst = out_t[:, off:off + ng * w_cols].rearrange(
                        "p (g q) -> p g q", g=ng)
                    # in0: group g reads x[:, D*(i+g) : D*(i+g)+w_cols]
                    # (overlapping windows -> explicit [step, count] AP)
                    b0 = xt[:, D * i:D * i + w_cols]
                    src = AP(b0.tensor, b0.offset,
                             [list(b0.ap[0]), [D, ng], [1, w_cols]])
                    # in1: proj group block, broadcast over the k positions
                    p0 = proj[:, D * i:D * (i + 1)]
                    bcast = AP(p0.tensor, p0.offset,
                               [list(p0.ap[0]), [D, ng], [0, w_cols // D],
                                [1, D]])
                    nc.vector.tensor_mul(dst, src, bcast)
                    nxt = i + 2
                    if nxt in SPLITS or nxt >= N - 1:
                        hi = GOFF[nxt] * D if nxt < N - 1 else FOUT
                        nc.sync.dma_start(
                            y_out[row0:row0 + PTILE, chunk_lo:hi],
                            out_t[:, chunk_lo:hi])
                        chunk_lo = hi

    nc.compile()
    return nc


def kernel(x: np.ndarray, W: np.ndarray) -> np.ndarray:
    assert x.shape == (B, N, D) and W.shape == (D, D)
    if "nc" not in _CACHE:
        _CACHE["nc"] = _build_nc()
    nc = _CACHE["nc"]

    xs = np.ascontiguousarray(x, dtype=np.float32).reshape(B, F)
    w = np.ascontiguousarray(W, dtype=np.float32)
    in_maps = [
        {"x": xs[c * BLOC:(c + 1) * BLOC], "w": w} for c in range(NCORES)
    ]
    res = run_bass_kernel_spmd(nc, in_maps, list(range(NCORES)))
    out = np.concatenate([res.results[c]["out"] for c in range(NCORES)],
                         axis=0)
    return out.astype(np.float32, copy=False)


# revision 21
# speedup vs baseline: 1.0219x; 1.0219x over previous
"""Trainium2 Bass kernel for nn_BiInteraction.

Reference computation:
    x: [B=8192, N=34, D=16] f32, W: [D, D] f32
    proj = einsum('bnd,de->bne', x, W)
    pairs (i, j) for i in [0, N-2], j in [i, N-1]  -> P = 594 pairs
    out[:, p, :] = proj[:, i_p, :] * x[:, j_p, :]  -> reshape [B, P*D = 9504]

Sharding: data-parallel over batch, 1024 rows per core, 8 cores.

Per-core kernel (per 128-batch tile; all stages pipelined by Tile):
  1. DMA x tile [128, 544] (batch on partitions, (n,d) flattened free dim);
     all 8 x tiles are prefetched up front so input DMAs never queue
     behind output DMAs
  2. Per 128-col block c: TensorE transpose -> ScalarE copy to SBUF ->
     TensorE matmul(lhsT=xT_block, rhs=W_blockdiag) -> ScalarE copy, giving
     proj[b, (n e)] in batch-major layout.  W_blockdiag [128,128] has W on
     the 16x16 diagonal blocks, so the contraction over rows (n', d)
     reduces over d only, separately per field n.
  3. VectorE: pair-groups: group i covers output pairs (i, j) for
     j in [i, 33] — a contiguous x slice times a broadcast proj block.
     Two adjacent groups are fused into ONE tensor_mul via explicit
     [step, count] access patterns (overlapping x windows, group i+1
     padded to group i's width; the D-column garbage spill into group
     i+2's start is rewritten by the next pair before any DMA reads it).
  4. Output [128, 9504] is DMA'd in 6 column chunks as the pairs finish
     (plus a K=32 proj fast path for fields 0-1), so the store stream
     starts ~5us into the kernel and stays saturated.
"""

import numpy as np

import concourse.bacc as bacc
import concourse.tile as tile
import concourse.mybir as mybir
from concourse import masks
from concourse.bass_types import AP
from concourse.bass_utils import run_bass_kernel_spmd

B, N, D = 8192, 34, 16
NCORES = 8
BLOC = B // NCORES            # 1024 rows per core
PTILE = 128                   # batch rows per tile (SBUF partitions)
NTILES = BLOC // PTILE        # 8
F = N * D                     # 544
F_PAD = F + D                 # pair-TT overlap pad
NPAIR = N * (N + 1) // 2 - 1  # 594
FOUT = NPAIR * D              # 9504

# group i covers pairs (i, j) for j in [i, N-1]; GOFF[i] = first pair index
GOFF = [0] * (N - 1)
for _i in range(1, N - 1):
    GOFF[_i] = GOFF[_i - 1] + (N - _i + 1)

_CACHE = {}


def _build_nc(repeat: int = 1):
    nc = bacc.Bacc("TRN2", target_bir_lowering=False, debug=False,
                   num_devices=NCORES)
    x_in = nc.dram_tensor("x", [BLOC, F], mybir.dt.float32,
                          kind="ExternalInput").ap()
    w_in = nc.dram_tensor("w", [D, D], mybir.dt.float32,
                          kind="ExternalInput").ap()
    y_out = nc.dram_tensor("out", [BLOC, FOUT], mybir.dt.float32,
                           kind="ExternalOutput").ap()

    f32 = mybir.dt.float32
    with tile.TileContext(nc) as tc:
        with (
            tc.tile_pool(name="const", bufs=1) as const_pool,
            tc.tile_pool(name="x", bufs=8) as x_pool,
            tc.tile_pool(name="xT_ps", bufs=2, space="PSUM") as xT_ps_pool,
            tc.tile_pool(name="xT_sb", bufs=2) as xT_sb_pool,
            tc.tile_pool(name="proj_ps", bufs=2, space="PSUM") as proj_ps_pool,
            tc.tile_pool(name="proj_sb", bufs=2) as proj_sb_pool,
            tc.tile_pool(name="out", bufs=4) as out_pool,
        ):
            # tile 0's x absolutely first (no deps), then the 8 tiny
            # block-diagonal-W DMAs while the DMA engines are idle anyway
            xts = []
            xt0 = x_pool.tile([PTILE, F_PAD], f32, tag="xt")
            nc.sync.dma_start(xt0[:, 0:F], x_in[0:PTILE, :])
            xts.append(xt0)
            wbd = const_pool.tile([128, 128], f32)
            nc.gpsimd.memset(wbd[:], 0.0)
            for n in range(8):
                nc.sync.dma_start(wbd[16 * n:16 * n + 16, 16 * n:16 * n + 16],
                                  w_in[:, :])
            ident = const_pool.tile([128, 128], f32)
            masks.make_identity(nc, ident[:])
            # dummy copy pulls the one-time ACT table load off the
            # critical path
            warm = const_pool.tile([1, 2], f32)
            nc.gpsimd.memset(warm[:], 0.0)
            nc.scalar.copy(warm[0:1, 1:2], warm[0:1, 0:1])

            # prefetch remaining x tiles (x is tiny: 17KB/partition total)
            for t in range(1, NTILES):
                xt = x_pool.tile([PTILE, F_PAD], f32, tag="xt")
                nc.gpsimd.dma_start(xt[:, 0:F], x_in[t * PTILE:(t + 1) * PTILE, :])
                xts.append(xt)

            # output DMA split points (group indices); last chunk small to
            # shrink the pipeline tail
            SPLITS = [2, 4, 8, 16, 24]

            for t in range(repeat * NTILES):
                xt = xts[t % NTILES]
                row0 = (t % NTILES) * PTILE

                # per 128-col block c: transpose -> copy -> proj matmul ->
                # copy, so group TTs for fields 8c..8c+7 start early
                xT_ps = xT_ps_pool.tile([128, 5 * 128], f32)
                xT = xT_sb_pool.tile([128, 5 * 128], f32)
                proj_ps = proj_ps_pool.tile([PTILE, F], f32)
                proj = proj_sb_pool.tile([PTILE, F], f32)
                for c in range(4):
                    nc.tensor.transpose(xT_ps[:, 128 * c:128 * (c + 1)],
                                        xt[:, 128 * c:128 * (c + 1)],
                                        ident[:])
                    nc.scalar.copy(xT[:, 128 * c:128 * (c + 1)],
                                   xT_ps[:, 128 * c:128 * (c + 1)])
                    if c == 0:
                        # K=32 fast path for fields 0-1: only needs the
                        # first two W diagonal blocks (wbd rows/cols 0:32),
                        # so the first pair-TT and output chunk start early
                        nc.tensor.matmul(proj_ps[:, 0:32],
                                         lhsT=xT[0:32, 0:128],
                                         rhs=wbd[0:32, 0:32],
                                         start=True, stop=True)
                        nc.scalar.copy(proj[:, 0:32], proj_ps[:, 0:32])
                        nc.tensor.matmul(proj_ps[:, 32:128],
                                         lhsT=xT[:, 0:128],
                                         rhs=wbd[:, 32:128],
                                         start=True, stop=True)
                        nc.scalar.copy(proj[:, 32:128], proj_ps[:, 32:128])
                        continue
                    nc.tensor.matmul(proj_ps[:, 128 * c:128 * (c + 1)],
                                     lhsT=xT[:, 128 * c:128 * (c + 1)],
                                     rhs=wbd[:], start=True, stop=True)
                    nc.scalar.copy(proj[:, 128 * c:128 * (c + 1)],
                                   proj_ps[:, 128 * c:128 * (c + 1)])
                nc.tensor.transpose(xT_ps[0:32, 512:640], xt[:, 512:544],
                                    ident[:])
                nc.scalar.copy(xT[0:32, 512:640], xT_ps[0:32, 512:640])
                nc.tensor.matmul(proj_ps[:, 512:544],
                                 lhsT=xT[0:32, 512:640],
                                 rhs=wbd[0:32, 0:32], start=True, stop=True)
                nc.scalar.copy(proj[:, 512:544], proj_ps[:, 512:544])

                # pairwise products: one broadcast tensor_mul per PAIR of
                # groups (i, i+1), group i+1 padded to group i's width. The
                # pad overwrites the first D cols of group i+2 with garbage,
                # which the next pair's TT rewrites before any DMA (all
                # SPLITS are even groups). DMA out finished chunks as we go.
                out_t = out_pool.tile([PTILE, FOUT], f32)
                chunk_lo = 0
                for i in range(0, N - 1, 2):
                    w_cols = (N - i) * D     # padded per-group width
                    ng = 2 if i + 1 < N - 1 else 1
                    off = GOFF[i] * D
                    dst = out_t[:, off:off + ng * w_cols].rearrange(
                        "p (g q) -> p g q", g=ng)
                    # in0: group g reads x[:, D*(i+g) : D*(i+g)+w_cols]
                    # (overlapping windows -> explicit [step, count] AP)
                    b0 = xt[:, D * i:D * i + w_cols]
                    src = AP(b0.tensor, b0.offset,
                             [list(b0.ap[0]), [D, ng], [1, w_cols]])
                    # in1: proj group block, broadcast over the k positions
                    p0 = proj[:, D * i:D * (i + 1)]
                    bcast = AP(p0.tensor, p0.offset,
                               [list(p0.ap[0]), [D, ng], [0, w_cols // D],
                                [1, D]])
                    nc.vector.tensor_mul(dst, src, bcast)
                    nxt = i + 2
                    if nxt in SPLITS or nxt >= N - 1:
                        hi = GOFF[nxt] * D if nxt < N - 1 else FOUT
                        nc.sync.dma_start(
                            y_out[row0:row0 + PTILE, chunk_lo:hi],
                            out_t[:, chunk_lo:hi])
                        chunk_lo = hi

    nc.compile()
    return nc


def kernel(x: np.ndarray, W: np.ndarray) -> np.ndarray:
    assert x.shape == (B, N, D) and W.shape == (D, D)
    if "nc" not in _CACHE:
        _CACHE["nc"] = _build_nc()
    nc = _CACHE["nc"]

    xs = np.ascontiguousarray(x, dtype=np.float32).reshape(B, F)
    w = np.ascontiguousarray(W, dtype=np.float32)
    in_maps = [
        {"x": xs[c * BLOC:(c + 1) * BLOC], "w": w} for c in range(NCORES)
    ]
    res = run_bass_kernel_spmd(nc, in_maps, list(range(NCORES)))
    out = np.concatenate([res.results[c]["out"] for c in range(NCORES)],
                         axis=0)
    return out.astype(np.float32, copy=False)


# revision 23
# speedup vs baseline: 1.0521x; 1.0295x over previous
"""Trainium2 Bass kernel for nn_BiInteraction.

Reference computation:
    x: [B=8192, N=34, D=16] f32, W: [D, D] f32
    proj = einsum('bnd,de->bne', x, W)
    pairs (i, j) for i in [0, N-2], j in [i, N-1]  -> P = 594 pairs
    out[:, p, :] = proj[:, i_p, :] * x[:, j_p, :]  -> reshape [B, P*D = 9504]

Sharding: data-parallel over batch, 1024 rows per core, 8 cores.

Per-core kernel (per 128-batch tile; all stages pipelined by Tile):
  1. DMA x tile [128, 544] (batch on partitions, (n,d) flattened free dim);
     all 8 x tiles are prefetched up front so input DMAs never queue
     behind output DMAs
  2. Per 128-col block c: TensorE transpose -> ScalarE copy to SBUF ->
     TensorE matmul(lhsT=xT_block, rhs=W_blockdiag) -> ScalarE copy, giving
     proj[b, (n e)] in batch-major layout.  W_blockdiag [128,128] has W on
     the 16x16 diagonal blocks, so the contraction over rows (n', d)
     reduces over d only, separately per field n.
  3. VectorE: pair-groups: group i covers output pairs (i, j) for
     j in [i, 33] — a contiguous x slice times a broadcast proj block.
     Two adjacent groups are fused into ONE tensor_mul via explicit
     [step, count] access patterns (overlapping x windows, group i+1
     padded to group i's width; the D-column garbage spill into group
     i+2's start is rewritten by the next pair before any DMA reads it).
  4. Output [128, 9504] is DMA'd in 6 column chunks as the pairs finish
     (plus a K=32 proj fast path for fields 0-1), so the store stream
     starts ~5us into the kernel and stays saturated.
"""

import numpy as np

import concourse.bacc as bacc
import concourse.tile as tile
import concourse.mybir as mybir
from concourse import masks
from concourse.bass_types import AP
from concourse.bass_utils import run_bass_kernel_spmd

B, N, D = 8192, 34, 16
NCORES = 8
BLOC = B // NCORES            # 1024 rows per core
PTILE = 128                   # batch rows per tile (SBUF partitions)
NTILES = BLOC // PTILE        # 8
F = N * D                     # 544
F_PAD = F + D                 # pair-TT overlap pad
NPAIR = N * (N + 1) // 2 - 1  # 594
FOUT = NPAIR * D              # 9504

# group i covers pairs (i, j) for j in [i, N-1]; GOFF[i] = first pair index
GOFF = [0] * (N - 1)
for _i in range(1, N - 1):
    GOFF[_i] = GOFF[_i - 1] + (N - _i + 1)

_CACHE = {}


def _build_nc(repeat: int = 1):
    nc = bacc.Bacc("TRN2", target_bir_lowering=False, debug=False,
                   num_devices=NCORES)
    x_in = nc.dram_tensor("x", [BLOC, F], mybir.dt.float32,
                          kind="ExternalInput").ap()
    w_in = nc.dram_tensor("w", [D, D], mybir.dt.float32,
                          kind="ExternalInput").ap()
    y_out = nc.dram_tensor("out", [BLOC, FOUT], mybir.dt.float32,
                           kind="ExternalOutput").ap()

    f32 = mybir.dt.float32
    with tile.TileContext(nc) as tc:
        with (
            tc.tile_pool(name="const", bufs=1) as const_pool,
            tc.tile_pool(name="x", bufs=8) as x_pool,
            tc.tile_pool(name="xT_ps", bufs=2, space="PSUM") as xT_ps_pool,
            tc.tile_pool(name="xT_sb", bufs=2) as xT_sb_pool,
            tc.tile_pool(name="proj_ps", bufs=2, space="PSUM") as proj_ps_pool,
            tc.tile_pool(name="proj_sb", bufs=3) as proj_sb_pool,
            tc.tile_pool(name="out_a", bufs=4) as out_a_pool,
            tc.tile_pool(name="out_b", bufs=4) as out_b_pool,
        ):
            # tile 0's x absolutely first (no deps), then the 8 tiny
            # block-diagonal-W DMAs while the DMA engines are idle anyway
            xts = []
            xt0 = x_pool.tile([PTILE, F_PAD], f32, tag="xt")
            nc.sync.dma_start(xt0[:, 0:F], x_in[0:PTILE, :])
            xts.append(xt0)
            wbd = const_pool.tile([128, 128], f32)
            nc.gpsimd.memset(wbd[:], 0.0)
            for n in range(8):
                nc.sync.dma_start(wbd[16 * n:16 * n + 16, 16 * n:16 * n + 16],
                                  w_in[:, :])
            ident = const_pool.tile([128, 128], f32)
            masks.make_identity(nc, ident[:])
            # dummy copy pulls the one-time ACT table load off the
            # critical path
            warm = const_pool.tile([1, 2], f32)
            nc.gpsimd.memset(warm[:], 0.0)
            nc.scalar.copy(warm[0:1, 1:2], warm[0:1, 0:1])

            # prefetch remaining x tiles (x is tiny: 17KB/partition total)
            for t in range(1, NTILES):
                xt = x_pool.tile([PTILE, F_PAD], f32, tag="xt")
                nc.gpsimd.dma_start(xt[:, 0:F], x_in[t * PTILE:(t + 1) * PTILE, :])
                xts.append(xt)

            # output DMA split points (group indices); last chunk small to
            # shrink the pipeline tail. HSPLIT is the half-tile boundary.
            SPLITS = [2, 4, 8, 16, 24]
            HSPLIT = 16
            HCOL = GOFF[HSPLIT] * D

            for t in range(repeat * NTILES):
                xt = xts[t % NTILES]
                row0 = (t % NTILES) * PTILE

                # per 128-col block c: transpose -> copy -> proj matmul ->
                # copy, so group TTs for fields 8c..8c+7 start early
                xT_ps = xT_ps_pool.tile([128, 5 * 128], f32)
                xT = xT_sb_pool.tile([128, 5 * 128], f32)
                proj_ps = proj_ps_pool.tile([PTILE, F], f32)
                proj = proj_sb_pool.tile([PTILE, F], f32)
                for c in range(4):
                    nc.tensor.transpose(xT_ps[:, 128 * c:128 * (c + 1)],
                                        xt[:, 128 * c:128 * (c + 1)],
                                        ident[:])
                    nc.scalar.copy(xT[:, 128 * c:128 * (c + 1)],
                                   xT_ps[:, 128 * c:128 * (c + 1)])
                    if c == 0:
                        # K=32 fast path for fields 0-1: only needs the
                        # first two W diagonal blocks (wbd rows/cols 0:32),
                        # so the first pair-TT and output chunk start early
                        nc.tensor.matmul(proj_ps[:, 0:32],
                                         lhsT=xT[0:32, 0:128],
                                         rhs=wbd[0:32, 0:32],
                                         start=True, stop=True)
                        nc.scalar.copy(proj[:, 0:32], proj_ps[:, 0:32])
                        nc.tensor.matmul(proj_ps[:, 32:128],
                                         lhsT=xT[:, 0:128],
                                         rhs=wbd[:, 32:128],
                                         start=True, stop=True)
                        nc.scalar.copy(proj[:, 32:128], proj_ps[:, 32:128])
                        continue
                    nc.tensor.matmul(proj_ps[:, 128 * c:128 * (c + 1)],
                                     lhsT=xT[:, 128 * c:128 * (c + 1)],
                                     rhs=wbd[:], start=True, stop=True)
                    nc.scalar.copy(proj[:, 128 * c:128 * (c + 1)],
                                   proj_ps[:, 128 * c:128 * (c + 1)])
                nc.tensor.transpose(xT_ps[0:32, 512:640], xt[:, 512:544],
                                    ident[:])
                nc.scalar.copy(xT[0:32, 512:640], xT_ps[0:32, 512:640])
                nc.tensor.matmul(proj_ps[:, 512:544],
                                 lhsT=xT[0:32, 512:640],
                                 rhs=wbd[0:32, 0:32], start=True, stop=True)
                nc.scalar.copy(proj[:, 512:544], proj_ps[:, 512:544])

                # pairwise products: one broadcast tensor_mul per PAIR of
                # groups (i, i+1), group i+1 padded to group i's width. The
                # pad overwrites the first D cols of group i+2 with garbage,
                # which the next pair's TT rewrites before any DMA (all
                # SPLITS are even groups). DMA out finished chunks as we go.
                # Output staged in two half tiles (split at group HSPLIT) so
                # buffer slots recycle at half-tile granularity; out_a has D
                # pad cols for the last pair's spill past the half boundary.
                out_a = out_a_pool.tile([PTILE, HCOL + D], f32)
                out_b = out_b_pool.tile([PTILE, FOUT - HCOL], f32)
                chunk_lo = 0
                for i in range(0, N - 1, 2):
                    w_cols = (N - i) * D     # padded per-group width
                    ng = 2 if i + 1 < N - 1 else 1
                    off = GOFF[i] * D
                    out_t, base = (out_a, 0) if i < HSPLIT else (out_b, HCOL)
                    dst = out_t[:, off - base:off - base + ng * w_cols] \
                        .rearrange("p (g q) -> p g q", g=ng)
                    # in0: group g reads x[:, D*(i+g) : D*(i+g)+w_cols]
                    # (overlapping windows -> explicit [step, count] AP)
                    b0 = xt[:, D * i:D * i + w_cols]
                    src = AP(b0.tensor, b0.offset,
                             [list(b0.ap[0]), [D, ng], [1, w_cols]])
                    # in1: proj group block, broadcast over the k positions
                    p0 = proj[:, D * i:D * (i + 1)]
                    bcast = AP(p0.tensor, p0.offset,
                               [list(p0.ap[0]), [D, ng], [0, w_cols // D],
                                [1, D]])
                    nc.vector.tensor_mul(dst, src, bcast)
                    nxt = i + 2
                    if nxt in SPLITS or nxt >= N - 1:
                        hi = GOFF[nxt] * D if nxt < N - 1 else FOUT
                        src_t, sbase = (out_a, 0) if i < HSPLIT else (out_b,
                                                                      HCOL)
                        nc.sync.dma_start(
                            y_out[row0:row0 + PTILE, chunk_lo:hi],
                            src_t[:, chunk_lo - sbase:hi - sbase])
                        chunk_lo = hi

    nc.compile()
    return nc


def kernel(x: np.ndarray, W: np.ndarray) -> np.ndarray:
    assert x.shape == (B, N, D) and W.shape == (D, D)
    if "nc" not in _CACHE:
        _CACHE["nc"] = _build_nc()
    nc = _CACHE["nc"]

    xs = np.ascontiguousarray(x, dtype=np.float32).reshape(B, F)
    w = np.ascontiguousarray(W, dtype=np.float32)
    in_maps = [
        {"x": xs[c * BLOC:(c + 1) * BLOC], "w": w} for c in range(NCORES)
    ]
    res = run_bass_kernel_spmd(nc, in_maps, list(range(NCORES)))
    out = np.concatenate([res.results[c]["out"] for c in range(NCORES)],
                         axis=0)
    return out.astype(np.float32, copy=False)
